# revision 20
# baseline (speedup 1.0000x reference)
"""Trainium2 Bass kernel for nn_AttentionBlock (B=2, L=2048, D=1024, H=16).

Sharding: tensor-parallel over heads. Each of 8 cores computes 2 heads:
Wq/Wk/Wv column-sharded, Wo row-sharded; host sums the 8 partial outputs.

v3: all matmul operands bf16 (fp32 moving operands stream at 2 cycles/col on
the PE; bf16 streams at 1 — measured), software-pipelined attention loop
(scores of tile jt issue before PV of tile jt-1 so the PE never head-of-line
blocks on the exp), PE-based V transpose, batched output DMA, bf16 partials.

Per-core dataflow:
  - x^T is prepared host-side ([D, B*L] bf16, layout prep only, no math).
  - qT/kT/vT = W.T @ xT   (weight-stationary, contraction over D)
  - vT is PE-transposed to v [L, dh] blocks with a ones column appended
    (aug), so the PV matmul also produces softmax denominators for free.
  - scoresT = kT.T @ qT per (head, batch) in [Lk, Lq] layout, two heads
    row-tiled concurrently on the PE; exp on ACT (no max-subtraction:
    scores ~ N(0,1), exp is fp32-safe);
  - aT += v_aug.T @ expT accumulates attention output (+ denominator row).
  - aT is normalized via a PE-broadcast reciprocal matrix.
  - out = aT.T @ Wo (heads accumulate in PSUM), written as bf16 partials.
"""
import numpy as np
from contextlib import ExitStack

import concourse.bacc as bacc
import concourse.tile as tile
import concourse.mybir as mybir
from concourse import bass_utils
from concourse.masks import make_identity

F32 = mybir.dt.float32
BF16 = mybir.dt.bfloat16
AF = mybir.ActivationFunctionType
ALU = mybir.AluOpType

B, L, D, H, DH = 2, 2048, 1024, 16, 64
NCORES = 8
HPC = H // NCORES       # heads per core
DHC = HPC * DH          # 128 = head-dim slice per core
KT = D // 128           # 8 k-tiles over the contraction dim


def build(Lb=L, debug=False):
    """Build the per-core Bass program for per-batch seq len Lb."""
    BLb = B * Lb
    NJT = Lb // 128            # key tiles per batch
    LC = min(512, Lb)          # query-chunk width
    NLC = Lb // LC             # query chunks per batch
    PC = min(512, BLb)         # projection chunk width
    NPC = BLb // PC            # projection chunks
    VB = 130                   # v block width per (b, jt): 2 heads x (64+ones)

    nc = bacc.Bacc("TRN2", target_bir_lowering=False, debug=debug, num_devices=8)

    xT = nc.dram_tensor("xT", [D, BLb], BF16, kind="ExternalInput")
    wq = nc.dram_tensor("wq", [D, DHC], BF16, kind="ExternalInput")
    wk = nc.dram_tensor("wk", [D, DHC], BF16, kind="ExternalInput")
    wv = nc.dram_tensor("wv", [D, DHC], BF16, kind="ExternalInput")
    wo = nc.dram_tensor("wo", [DHC, D], BF16, kind="ExternalInput")
    bq = nc.dram_tensor("bq", [DHC, 1], F32, kind="ExternalInput")
    bk = nc.dram_tensor("bk", [DHC, 1], F32, kind="ExternalInput")
    bv = nc.dram_tensor("bv", [DHC, 1], F32, kind="ExternalInput")
    out = nc.dram_tensor("out", [BLb, D], BF16, kind="ExternalOutput")

    xT_v = xT.ap().rearrange("(kt p) l -> p kt l", p=128)   # [128, KT, BLb]
    wq_v = wq.ap().rearrange("(kt p) m -> p kt m", p=128)   # [128, KT, DHC]
    wk_v = wk.ap().rearrange("(kt p) m -> p kt m", p=128)
    wv_v = wv.ap().rearrange("(kt p) m -> p kt m", p=128)

    with tile.TileContext(nc) as tc, ExitStack() as ctx:
        # --- pools ---
        persist = ctx.enter_context(tc.tile_pool(name="persist", bufs=1))
        xpool = ctx.enter_context(tc.tile_pool(name="xchunk", bufs=3))
        vstage = ctx.enter_context(tc.tile_pool(name="vstage", bufs=2))
        stpool = ctx.enter_context(tc.tile_pool(name="stpool", bufs=4))
        expool = ctx.enter_context(tc.tile_pool(name="expool", bufs=3))
        denpool = ctx.enter_context(tc.tile_pool(name="denpool", bufs=2))
        outpool = ctx.enter_context(tc.tile_pool(name="outpool", bufs=2))
        # PSUM budget (8 banks): sc 2x2 + acc 1x2 + psing 2x1 -> 8
        scpool = ctx.enter_context(tc.tile_pool(name="scpool", bufs=2, space="PSUM"))
        accpool = ctx.enter_context(tc.tile_pool(name="accpool", bufs=1, space="PSUM"))
        psing = ctx.enter_context(tc.tile_pool(name="psing", bufs=2, space="PSUM"))

        # --- persistent tiles ---
        qT_sb = persist.tile([128, BLb], BF16, tag="qT")
        kT_sb = persist.tile([128, BLb], BF16, tag="kT")
        v_sb = persist.tile([128, B * NJT * VB], BF16, tag="v")
        aT_sb = [
            persist.tile([128, Lb], BF16, tag=f"aT{b}", name=f"aT{b}")
            for b in range(B)
        ]
        wq_sb = persist.tile([128, KT, DHC], BF16, tag="wq")
        wk_sb = persist.tile([128, KT, DHC], BF16, tag="wk")
        wv_sb = persist.tile([128, KT, DHC], BF16, tag="wv")
        wo_sb = persist.tile([DHC, D], BF16, tag="wo")
        bq_sb = persist.tile([DHC, 1], F32, tag="bq")
        bk_sb = persist.tile([DHC, 1], F32, tag="bk")
        bv_sb = persist.tile([DHC, 1], F32, tag="bv")
        ident = persist.tile([128, 128], BF16, tag="ident")
        ones64 = persist.tile([65, 64], BF16, tag="ones64")  # row 64 used

        # --- phase A: loads & constants ---
        nc.sync.dma_start(wq_sb[:], wq_v)
        make_identity(nc, ident[:])
        # (wk/wv/wo/biases are issued on the scalar queue inside the first
        # projection chunk so their issue overlaps the x loads)
        nc.vector.memset(ones64[:], 1.0)
        # ones-columns of v (aug denominator trick)
        v_cols = v_sb[:].rearrange("p (n c) -> p n c", c=65)
        nc.vector.memset(v_cols[:, :, 64:65], 1.0)

        # --- phase emitters ---
        def emit_proj_chunk(chn):
            """Projections for rows [chn*PC, (chn+1)*PC) + v transpose."""
            cs = chn * PC
            xt = xpool.tile([128, KT, PC], BF16, tag="xt")
            if chn == 0:
                # first chunk: 2-ktile pieces on the sync queue while the
                # remaining weights/biases issue in parallel on the scalar
                # queue (idle at startup) so DMA issue time is not serial
                for kt in range(0, KT, 2):
                    nc.sync.dma_start(
                        xt[:, kt:kt + 2, :], xT_v[:, kt:kt + 2, cs:cs + PC]
                    )
                nc.scalar.dma_start(bq_sb[:], bq.ap())
                nc.scalar.dma_start(wk_sb[:], wk_v)
                nc.scalar.dma_start(bk_sb[:], bk.ap())
                nc.scalar.dma_start(wv_sb[:], wv_v)
                nc.scalar.dma_start(bv_sb[:], bv.ap())
                nc.scalar.dma_start(wo_sb[:], wo.ap())
            else:
                nc.sync.dma_start(xt[:], xT_v[:, :, cs:cs + PC])

            for w_sb, b_sb, dst in ((wq_sb, bq_sb, qT_sb), (wk_sb, bk_sb, kT_sb)):
                ps = psing.tile([128, PC], F32, tag="single")
                for kt in range(KT):
                    nc.tensor.matmul(
                        ps[:, :], w_sb[:, kt, :], xt[:, kt, :],
                        start=(kt == 0), stop=(kt == KT - 1),
                    )
                nc.vector.tensor_scalar(
                    dst[:, cs:cs + PC], ps[:, :], b_sb[:, 0:1], None, ALU.add
                )

            # v chunk -> staging (vT layout), then PE transpose into v_sb
            ps = psing.tile([128, PC], F32, tag="single")
            for kt in range(KT):
                nc.tensor.matmul(
                    ps[:, :], wv_sb[:, kt, :], xt[:, kt, :],
                    start=(kt == 0), stop=(kt == KT - 1),
                )
            vt = vstage.tile([128, PC], BF16, tag="vt")
            nc.vector.tensor_scalar(vt[:], ps[:, :], bv_sb[:, 0:1], None, ALU.add)

            for jl in range(PC // 128):
                gl = cs + jl * 128          # global row offset in [0, BLb)
                b_idx, jt = gl // Lb, (gl % Lb) // 128
                blk = (b_idx * NJT + jt) * VB
                # DMA-xbar transpose (sync queue, not the PE): one [128,128]
                # block covers both heads: out[j, h*64+d]
                st = stpool.tile([128, 128], BF16, tag="st", name="st")
                nc.sync.dma_start_transpose(
                    st[:, :], vt[:, jl * 128:(jl + 1) * 128]
                )
                vdst = v_sb[:, blk:blk + VB].rearrange("p (h c) -> p h c", h=2)
                stv = st[:].rearrange("p (h c) -> p h c", h=2)
                nc.vector.tensor_copy(vdst[:, :, 0:DH], stv[:, :, :])

        def make_drain(b, qo, lcw, aT):
            """Deferred drain for chunk (b, qo:qo+lcw): a list of closures
            emitted one per jt-slot of the NEXT attention chunk (or flushed
            at the end). Split so no piece hogs the PE queue."""
            st = {}

            def d_copies():
                den = denpool.tile([65, 2, lcw], BF16, tag="den",
                                   padded_shape=[65, 2, 512], name="den")
                nc.vector.tensor_copy(den[64:65, :, :], aT[64:65, :, :])
                st["den"] = den
                for h in range(HPC):
                    nc.vector.tensor_copy(
                        aT_sb[b][h * DH:(h + 1) * DH, qo:qo + lcw],
                        aT[0:DH, h, :],
                    )

            def d_rep():
                rep = psing.tile([128, lcw], F32, tag="single",
                                 padded_shape=[128, 512], name="rep")
                for h in range(HPC):
                    nc.tensor.matmul(
                        rep[h * DH:(h + 1) * DH, :],
                        ones64[64:65, :],
                        st["den"][64:65, h, :],
                        start=True, stop=True,
                        tile_position=(64, h * DH),
                    )
                st["rep"] = rep
                st["rrecb"] = denpool.tile([128, lcw], BF16, tag="rrecb",
                                           padded_shape=[128, 512], name="rrecb")

            def d_recip(half):
                def f():
                    hw = lcw // 2
                    cols = slice(half * hw, (half + 1) * hw)
                    with nc.allow_low_precision(reason="softmax denom, tol 2e-2"):
                        nc.vector.reciprocal(
                            st["rrecb"][:, cols], st["rep"][:, cols]
                        )
                return f

            def d_mul(half):
                def f():
                    hw = lcw // 2
                    cols = slice(half * hw, (half + 1) * hw)
                    gcols = slice(qo + half * hw, qo + (half + 1) * hw)
                    nc.vector.tensor_mul(
                        aT_sb[b][:, gcols], aT_sb[b][:, gcols],
                        st["rrecb"][:, cols],
                    )
                    if half == 0:
                        st["ot"] = outpool.tile(
                            [128, lcw // 128, D], BF16, tag="ot",
                            padded_shape=[128, 4, D], name=f"ot{b}_{qo}"
                        )
                return f

            def d_po(ti):
                def f():
                    t = qo // 128 + ti
                    for nch in range(2):
                        po = psing.tile([128, 512], F32, tag="single")
                        nc.tensor.matmul(
                            po[:, :],
                            aT_sb[b][:, t * 128:(t + 1) * 128],
                            wo_sb[:, nch * 512:(nch + 1) * 512],
                            start=True, stop=True,
                        )
                        nc.vector.tensor_copy(
                            st["ot"][:, ti, nch * 512:(nch + 1) * 512], po[:, :]
                        )
                return f

            def d_dma():
                out_rows = out.ap()[b * Lb + qo:b * Lb + qo + lcw, :]
                nc.sync.dma_start(
                    out_rows.rearrange("(t p) d -> p t d", p=128), st["ot"][:]
                )

            skip = lambda: None
            pieces = [d_copies, d_rep, d_recip(0), d_mul(0), d_recip(1),
                      d_mul(1), skip]
            for ti in range(lcw // 128):
                pieces += [d_po(ti), skip]
            pieces.append(d_dma)
            return pieces

        def emit_att_chunk(b, qo, lcw, deferred):
            """Attention for query columns [qo, qo+lcw) of batch b, software-
            pipelined 2 deep: scores/exp of tile jt issue 2 slots ahead of
            its PV, and the previous chunk's drain pieces interleave into
            the early slots."""
            q0 = b * Lb + qo
            aT = accpool.tile([65, 2, lcw], F32, tag="acc",
                              padded_shape=[65, 2, 512], name="acc")
            exq = []
            for jt in range(NJT + 2):
                if jt < NJT:
                    k0 = b * Lb + jt * 128
                    sc = scpool.tile([128, 2, lcw], F32, tag="sc",
                                     padded_shape=[128, 2, 512], name="sc")
                    for h in range(HPC):
                        nc.tensor.matmul(
                            sc[:, h, :],
                            kT_sb[h * DH:(h + 1) * DH, k0:k0 + 128],
                            qT_sb[h * DH:(h + 1) * DH, q0:q0 + lcw],
                            start=True, stop=True,
                            tile_position=(h * DH, 0),
                        )
                    ex = expool.tile([128, 2, lcw], BF16, tag="ex",
                                     padded_shape=[128, 2, 512], name="ex")
                    nc.scalar.activation(ex[:], sc[:], AF.Exp)
                    exq.append(ex)
                if jt < len(deferred):
                    deferred[jt]()
                if jt >= 2:
                    pj = jt - 2
                    blk = (b * NJT + pj) * VB
                    for h in range(HPC):
                        nc.tensor.matmul(
                            aT[:, h, :],
                            v_sb[:, blk + h * 65:blk + h * 65 + 65],
                            exq[pj][:, h, :],
                            start=(pj == 0), stop=(pj == NJT - 1),
                        )
            return make_drain(b, qo, lcw, aT)

        # --- main schedule: proj b=0, then attention interleaved with the
        # remaining projection chunks, drains deferred one chunk back; the
        # final chunk is split in half so its drain tail is shorter ---
        att_chunks = []
        for b in range(B):
            for lc in range(NLC):
                if b == B - 1 and lc == NLC - 1:
                    att_chunks.append((b, lc * LC, LC // 2))
                    att_chunks.append((b, lc * LC + LC // 2, LC // 2))
                else:
                    att_chunks.append((b, lc * LC, LC))
        proj_b0 = list(range(NLC))           # chunks covering batch 0 rows
        proj_rest = list(range(NLC, NPC))
        for chn in proj_b0:
            emit_proj_chunk(chn)
        deferred = []
        for ci, (b, qo, lcw) in enumerate(att_chunks):
            deferred = emit_att_chunk(b, qo, lcw, deferred)
            if ci < len(proj_rest):
                emit_proj_chunk(proj_rest[ci])
        for piece in deferred:
            piece()

    nc.compile()
    return nc


_NC_CACHE = {}


def _get_nc(Lb=L):
    if Lb not in _NC_CACHE:
        _NC_CACHE[Lb] = build(Lb)
    return _NC_CACHE[Lb]


def make_in_maps(x, Wq, bq, Wk, bk, Wv, bv, Wo, bo, Lb=L):
    import ml_dtypes
    bf16 = ml_dtypes.bfloat16
    s = np.float32(DH ** (-0.25))
    BLb = B * Lb
    xT = np.ascontiguousarray(
        np.asarray(x, np.float32).reshape(BLb, D).T
    ).astype(bf16)
    Wq, Wk, Wv, Wo = (np.asarray(a, np.float32) for a in (Wq, Wk, Wv, Wo))
    bq, bk, bv = (np.asarray(a, np.float32) for a in (bq, bk, bv))
    in_maps = []
    for c in range(NCORES):
        hs = slice(c * DHC, (c + 1) * DHC)
        in_maps.append({
            "xT": xT,
            "wq": np.ascontiguousarray(Wq[:, hs] * s).astype(bf16),
            "wk": np.ascontiguousarray(Wk[:, hs] * s).astype(bf16),
            "wv": np.ascontiguousarray(Wv[:, hs]).astype(bf16),
            "wo": np.ascontiguousarray(Wo[hs, :]).astype(bf16),
            "bq": np.ascontiguousarray((bq[hs] * s).reshape(DHC, 1)),
            "bk": np.ascontiguousarray((bk[hs] * s).reshape(DHC, 1)),
            "bv": np.ascontiguousarray(bv[hs].reshape(DHC, 1)),
        })
    return in_maps


def kernel(x, Wq, bq, Wk, bk, Wv, bv, Wo, bo, **run_kwargs):
    x = np.asarray(x, np.float32)
    nc = _get_nc(L)
    in_maps = make_in_maps(x, Wq, bq, Wk, bk, Wv, bv, Wo, bo, L)
    res = bass_utils.run_bass_kernel_spmd(nc, in_maps, list(range(NCORES)), **run_kwargs)
    acc = np.zeros((B * L, D), np.float32)
    for r in res.results:
        acc += np.asarray(r["out"], np.float32)
    acc += np.asarray(bo, np.float32)[None, :]
    out = acc.reshape(B, L, D)
    kernel.last_results = res
    return out


# revision 21
# speedup vs baseline: 1.1262x; 1.1262x over previous
"""Trainium2 Bass kernel for nn_AttentionBlock (B=2, L=2048, D=1024, H=16).

Sharding: tensor-parallel over heads. Each of 8 cores computes 2 heads:
Wq/Wk/Wv column-sharded, Wo row-sharded; host sums the 8 partial outputs.

v3: all matmul operands bf16 (fp32 moving operands stream at 2 cycles/col on
the PE; bf16 streams at 1 — measured), software-pipelined attention loop
(scores of tile jt issue before PV of tile jt-1 so the PE never head-of-line
blocks on the exp), PE-based V transpose, batched output DMA, bf16 partials.

Per-core dataflow:
  - x^T is prepared host-side ([D, B*L] bf16, layout prep only, no math).
  - qT/kT/vT = W.T @ xT   (weight-stationary, contraction over D)
  - vT is PE-transposed to v [L, dh] blocks with a ones column appended
    (aug), so the PV matmul also produces softmax denominators for free.
  - scoresT = kT.T @ qT per (head, batch) in [Lk, Lq] layout, two heads
    row-tiled concurrently on the PE; exp on ACT (no max-subtraction:
    scores ~ N(0,1), exp is fp32-safe);
  - aT += v_aug.T @ expT accumulates attention output (+ denominator row).
  - aT is normalized via a PE-broadcast reciprocal matrix.
  - out = aT.T @ Wo (heads accumulate in PSUM), written as bf16 partials.
"""
import numpy as np
from contextlib import ExitStack

import concourse.bacc as bacc
import concourse.tile as tile
import concourse.mybir as mybir
from concourse import bass_utils
from concourse.masks import make_identity

F32 = mybir.dt.float32
BF16 = mybir.dt.bfloat16
AF = mybir.ActivationFunctionType
ALU = mybir.AluOpType

B, L, D, H, DH = 2, 2048, 1024, 16, 64
NCORES = 8
HPC = H // NCORES       # heads per core
DHC = HPC * DH          # 128 = head-dim slice per core
KT = D // 128           # 8 k-tiles over the contraction dim


def build(Lb=L, debug=False):
    """Build the per-core Bass program for per-batch seq len Lb."""
    BLb = B * Lb
    NJT = Lb // 128            # key tiles per batch
    LC = min(512, Lb)          # query-chunk width
    NLC = Lb // LC             # query chunks per batch
    PC = min(512, BLb)         # projection chunk width
    NPC = BLb // PC            # projection chunks
    VB = 130                   # v block width per (b, jt): 2 heads x (64+ones)

    nc = bacc.Bacc("TRN2", target_bir_lowering=False, debug=debug, num_devices=8)

    xT = nc.dram_tensor("xT", [D, BLb], BF16, kind="ExternalInput")
    wq = nc.dram_tensor("wq", [D, DHC], BF16, kind="ExternalInput")
    wk = nc.dram_tensor("wk", [D, DHC], BF16, kind="ExternalInput")
    wv = nc.dram_tensor("wv", [D, DHC], BF16, kind="ExternalInput")
    wo = nc.dram_tensor("wo", [DHC, D], BF16, kind="ExternalInput")
    bq = nc.dram_tensor("bq", [DHC, 1], F32, kind="ExternalInput")
    bk = nc.dram_tensor("bk", [DHC, 1], F32, kind="ExternalInput")
    bv = nc.dram_tensor("bv", [DHC, 1], F32, kind="ExternalInput")
    out = nc.dram_tensor("out", [BLb, D], BF16, kind="ExternalOutput")

    xT_v = xT.ap().rearrange("(kt p) l -> p kt l", p=128)   # [128, KT, BLb]
    wq_v = wq.ap().rearrange("(kt p) m -> p kt m", p=128)   # [128, KT, DHC]
    wk_v = wk.ap().rearrange("(kt p) m -> p kt m", p=128)
    wv_v = wv.ap().rearrange("(kt p) m -> p kt m", p=128)

    with tile.TileContext(nc) as tc, ExitStack() as ctx:
        # --- pools ---
        persist = ctx.enter_context(tc.tile_pool(name="persist", bufs=1))
        xpool = ctx.enter_context(tc.tile_pool(name="xchunk", bufs=3))
        vstage = ctx.enter_context(tc.tile_pool(name="vstage", bufs=2))
        expool = ctx.enter_context(tc.tile_pool(name="expool", bufs=3))
        denpool = ctx.enter_context(tc.tile_pool(name="denpool", bufs=2))
        outpool = ctx.enter_context(tc.tile_pool(name="outpool", bufs=2))
        # PSUM budget (8 banks): sc 2x2 + acc 1x2 + psing 2x1 -> 8
        scpool = ctx.enter_context(tc.tile_pool(name="scpool", bufs=2, space="PSUM"))
        accpool = ctx.enter_context(tc.tile_pool(name="accpool", bufs=1, space="PSUM"))
        psing = ctx.enter_context(tc.tile_pool(name="psing", bufs=2, space="PSUM"))

        # --- persistent tiles ---
        qT_sb = persist.tile([128, BLb], BF16, tag="qT")
        kT_sb = persist.tile([128, BLb], BF16, tag="kT")
        v_sb = persist.tile([128, B * NJT * VB], BF16, tag="v")
        aT_sb = [
            persist.tile([128, Lb], BF16, tag=f"aT{b}", name=f"aT{b}")
            for b in range(B)
        ]
        wq_sb = persist.tile([128, KT, DHC], BF16, tag="wq")
        wk_sb = persist.tile([128, KT, DHC], BF16, tag="wk")
        wv_sb = persist.tile([128, KT, DHC], BF16, tag="wv")
        wo_sb = persist.tile([DHC, D], BF16, tag="wo")
        bq_sb = persist.tile([DHC, 1], F32, tag="bq")
        bk_sb = persist.tile([DHC, 1], F32, tag="bk")
        bv_sb = persist.tile([DHC, 1], F32, tag="bv")
        ident = persist.tile([128, 128], BF16, tag="ident")
        ones64 = persist.tile([65, 64], BF16, tag="ones64")  # row 64 used

        # --- phase A: loads & constants ---
        nc.sync.dma_start(wq_sb[:], wq_v)
        make_identity(nc, ident[:])
        # (wk/wv/wo/biases are issued on the scalar queue inside the first
        # projection chunk so their issue overlaps the x loads)
        nc.vector.memset(ones64[:], 1.0)
        # ones-columns of v (aug denominator trick)
        v_cols = v_sb[:].rearrange("p (n c) -> p n c", c=65)
        nc.vector.memset(v_cols[:, :, 64:65], 1.0)
        # HAM warmup: the PE clock sits gated at 1.2 GHz until ~3.4us of
        # sustained activity; spin dummy matmuls on the identity tile while
        # the first x/weight DMAs are in flight so the projection phase
        # starts at the full 2.4 GHz
        warm = scpool.tile([128, 2, LC], F32, tag="sc",
                           padded_shape=[128, 2, 512], name="warm")
        for _ in range(40):
            nc.tensor.matmul(
                warm[:, 0, 0:128], ident[:], ident[:], start=True, stop=True
            )

        # --- phase emitters ---
        def emit_proj_chunk(chn):
            """Projections for rows [chn*PC, (chn+1)*PC) + v transpose."""
            cs = chn * PC
            xt = xpool.tile([128, KT, PC], BF16, tag="xt")
            if chn == 0:
                # first chunk: 2-ktile pieces on the sync queue while the
                # remaining weights/biases issue in parallel on the scalar
                # queue (idle at startup) so DMA issue time is not serial
                for kt in range(0, KT, 2):
                    nc.sync.dma_start(
                        xt[:, kt:kt + 2, :], xT_v[:, kt:kt + 2, cs:cs + PC]
                    )
                nc.scalar.dma_start(bq_sb[:], bq.ap())
                nc.scalar.dma_start(wk_sb[:], wk_v)
                nc.scalar.dma_start(bk_sb[:], bk.ap())
                nc.scalar.dma_start(wv_sb[:], wv_v)
                nc.scalar.dma_start(bv_sb[:], bv.ap())
                nc.scalar.dma_start(wo_sb[:], wo.ap())
            else:
                nc.sync.dma_start(xt[:], xT_v[:, :, cs:cs + PC])

            for w_sb, b_sb, dst in ((wq_sb, bq_sb, qT_sb), (wk_sb, bk_sb, kT_sb)):
                ps = psing.tile([128, PC], F32, tag="single")
                for kt in range(KT):
                    nc.tensor.matmul(
                        ps[:, :], w_sb[:, kt, :], xt[:, kt, :],
                        start=(kt == 0), stop=(kt == KT - 1),
                    )
                nc.vector.tensor_scalar(
                    dst[:, cs:cs + PC], ps[:, :], b_sb[:, 0:1], None, ALU.add
                )

            # v chunk -> staging (vT layout), then PE transpose into v_sb
            ps = psing.tile([128, PC], F32, tag="single")
            for kt in range(KT):
                nc.tensor.matmul(
                    ps[:, :], wv_sb[:, kt, :], xt[:, kt, :],
                    start=(kt == 0), stop=(kt == KT - 1),
                )
            vt = vstage.tile([128, PC], BF16, tag="vt")
            nc.vector.tensor_scalar(vt[:], ps[:, :], bv_sb[:, 0:1], None, ALU.add)

            for jp in range(PC // 256):
                gl = cs + jp * 256          # global row offset in [0, BLb)
                b_idx, jt = gl // Lb, (gl % Lb) // 128
                blk = (b_idx * NJT + jt) * VB
                pt = psing.tile([128, 2, 128], BF16, tag="single")
                # each [128,128] transpose covers both heads: out[j, h*64+d];
                # two consecutive key tiles share one psum tile and one copy
                for jl in range(2):
                    nc.tensor.transpose(
                        pt[:, jl, :],
                        vt[:, jp * 256 + jl * 128:jp * 256 + (jl + 1) * 128],
                        ident[:],
                    )
                vdst = v_sb[:, blk:blk + 2 * VB].rearrange(
                    "p (j h c) -> p j h c", j=2, c=65
                )
                ptv = pt[:].rearrange("p j (h c) -> p j h c", h=2)
                nc.vector.tensor_copy(vdst[:, :, :, 0:DH], ptv[:, :, :, :])

        def make_drain(b, qo, lcw, aT):
            """Deferred drain for chunk (b, qo:qo+lcw): a list of closures
            emitted one per jt-slot of the NEXT attention chunk (or flushed
            at the end). Split so no piece hogs the PE queue."""
            st = {}

            def d_copies():
                den = denpool.tile([65, 2, lcw], BF16, tag="den",
                                   padded_shape=[65, 2, 512], name="den")
                nc.vector.tensor_copy(den[64:65, :, :], aT[64:65, :, :])
                st["den"] = den
                for h in range(HPC):
                    nc.vector.tensor_copy(
                        aT_sb[b][h * DH:(h + 1) * DH, qo:qo + lcw],
                        aT[0:DH, h, :],
                    )

            def d_rep():
                rep = psing.tile([128, lcw], F32, tag="single",
                                 padded_shape=[128, 512], name="rep")
                for h in range(HPC):
                    nc.tensor.matmul(
                        rep[h * DH:(h + 1) * DH, :],
                        ones64[64:65, :],
                        st["den"][64:65, h, :],
                        start=True, stop=True,
                        tile_position=(64, h * DH),
                    )
                st["rep"] = rep
                st["rrecb"] = denpool.tile([128, lcw], BF16, tag="rrecb",
                                           padded_shape=[128, 512], name="rrecb")

            def d_recip(half):
                def f():
                    hw = lcw // 2
                    cols = slice(half * hw, (half + 1) * hw)
                    with nc.allow_low_precision(reason="softmax denom, tol 2e-2"):
                        nc.vector.reciprocal(
                            st["rrecb"][:, cols], st["rep"][:, cols]
                        )
                return f

            def d_mul(half):
                def f():
                    hw = lcw // 2
                    cols = slice(half * hw, (half + 1) * hw)
                    gcols = slice(qo + half * hw, qo + (half + 1) * hw)
                    nc.vector.tensor_mul(
                        aT_sb[b][:, gcols], aT_sb[b][:, gcols],
                        st["rrecb"][:, cols],
                    )
                    if half == 0:
                        st["ot"] = outpool.tile(
                            [128, lcw // 128, D], BF16, tag="ot",
                            padded_shape=[128, 4, D], name=f"ot{b}_{qo}"
                        )
                return f

            def d_po(ti):
                def f():
                    t = qo // 128 + ti
                    for nch in range(2):
                        po = psing.tile([128, 512], F32, tag="single")
                        nc.tensor.matmul(
                            po[:, :],
                            aT_sb[b][:, t * 128:(t + 1) * 128],
                            wo_sb[:, nch * 512:(nch + 1) * 512],
                            start=True, stop=True,
                        )
                        nc.vector.tensor_copy(
                            st["ot"][:, ti, nch * 512:(nch + 1) * 512], po[:, :]
                        )
                return f

            def d_dma():
                out_rows = out.ap()[b * Lb + qo:b * Lb + qo + lcw, :]
                nc.sync.dma_start(
                    out_rows.rearrange("(t p) d -> p t d", p=128), st["ot"][:]
                )

            skip = lambda: None
            pieces = [d_copies, d_rep, d_recip(0), d_mul(0), d_recip(1),
                      d_mul(1), skip]
            for ti in range(lcw // 128):
                pieces += [d_po(ti), skip]
            pieces.append(d_dma)
            return pieces

        def emit_att_chunk(b, qo, lcw, deferred):
            """Attention for query columns [qo, qo+lcw) of batch b, software-
            pipelined 2 deep: scores/exp of tile jt issue 2 slots ahead of
            its PV, and the previous chunk's drain pieces interleave into
            the early slots."""
            q0 = b * Lb + qo
            aT = accpool.tile([65, 2, lcw], F32, tag="acc",
                              padded_shape=[65, 2, 512], name="acc")
            exq = []
            for jt in range(NJT + 2):
                if jt < NJT:
                    k0 = b * Lb + jt * 128
                    sc = scpool.tile([128, 2, lcw], F32, tag="sc",
                                     padded_shape=[128, 2, 512], name="sc")
                    for h in range(HPC):
                        nc.tensor.matmul(
                            sc[:, h, :],
                            kT_sb[h * DH:(h + 1) * DH, k0:k0 + 128],
                            qT_sb[h * DH:(h + 1) * DH, q0:q0 + lcw],
                            start=True, stop=True,
                            tile_position=(h * DH, 0),
                        )
                    ex = expool.tile([128, 2, lcw], BF16, tag="ex",
                                     padded_shape=[128, 2, 512], name="ex")
                    nc.scalar.activation(ex[:], sc[:], AF.Exp)
                    exq.append(ex)
                if jt < len(deferred):
                    deferred[jt]()
                if jt >= 2:
                    pj = jt - 2
                    blk = (b * NJT + pj) * VB
                    for h in range(HPC):
                        nc.tensor.matmul(
                            aT[:, h, :],
                            v_sb[:, blk + h * 65:blk + h * 65 + 65],
                            exq[pj][:, h, :],
                            start=(pj == 0), stop=(pj == NJT - 1),
                        )
            return make_drain(b, qo, lcw, aT)

        # --- main schedule: proj b=0, then attention interleaved with the
        # remaining projection chunks, drains deferred one chunk back; the
        # final chunk is split in half so its drain tail is shorter ---
        att_chunks = []
        for b in range(B):
            for lc in range(NLC):
                if b == B - 1 and lc == NLC - 1:
                    att_chunks.append((b, lc * LC, LC // 2))
                    att_chunks.append((b, lc * LC + LC // 2, LC // 2))
                else:
                    att_chunks.append((b, lc * LC, LC))
        proj_b0 = list(range(NLC))           # chunks covering batch 0 rows
        proj_rest = list(range(NLC, NPC))
        for chn in proj_b0:
            emit_proj_chunk(chn)
        deferred = []
        for ci, (b, qo, lcw) in enumerate(att_chunks):
            deferred = emit_att_chunk(b, qo, lcw, deferred)
            if ci < len(proj_rest):
                emit_proj_chunk(proj_rest[ci])
        for piece in deferred:
            piece()

    nc.compile()
    return nc


_NC_CACHE = {}


def _get_nc(Lb=L):
    if Lb not in _NC_CACHE:
        _NC_CACHE[Lb] = build(Lb)
    return _NC_CACHE[Lb]


def make_in_maps(x, Wq, bq, Wk, bk, Wv, bv, Wo, bo, Lb=L):
    import ml_dtypes
    bf16 = ml_dtypes.bfloat16
    s = np.float32(DH ** (-0.25))
    BLb = B * Lb
    xT = np.ascontiguousarray(
        np.asarray(x, np.float32).reshape(BLb, D).T
    ).astype(bf16)
    Wq, Wk, Wv, Wo = (np.asarray(a, np.float32) for a in (Wq, Wk, Wv, Wo))
    bq, bk, bv = (np.asarray(a, np.float32) for a in (bq, bk, bv))
    in_maps = []
    for c in range(NCORES):
        hs = slice(c * DHC, (c + 1) * DHC)
        in_maps.append({
            "xT": xT,
            "wq": np.ascontiguousarray(Wq[:, hs] * s).astype(bf16),
            "wk": np.ascontiguousarray(Wk[:, hs] * s).astype(bf16),
            "wv": np.ascontiguousarray(Wv[:, hs]).astype(bf16),
            "wo": np.ascontiguousarray(Wo[hs, :]).astype(bf16),
            "bq": np.ascontiguousarray((bq[hs] * s).reshape(DHC, 1)),
            "bk": np.ascontiguousarray((bk[hs] * s).reshape(DHC, 1)),
            "bv": np.ascontiguousarray(bv[hs].reshape(DHC, 1)),
        })
    return in_maps


def kernel(x, Wq, bq, Wk, bk, Wv, bv, Wo, bo, **run_kwargs):
    x = np.asarray(x, np.float32)
    nc = _get_nc(L)
    in_maps = make_in_maps(x, Wq, bq, Wk, bk, Wv, bv, Wo, bo, L)
    res = bass_utils.run_bass_kernel_spmd(nc, in_maps, list(range(NCORES)), **run_kwargs)
    acc = np.zeros((B * L, D), np.float32)
    for r in res.results:
        acc += np.asarray(r["out"], np.float32)
    acc += np.asarray(bo, np.float32)[None, :]
    out = acc.reshape(B, L, D)
    kernel.last_results = res
    return out


# revision 22
# speedup vs baseline: 1.1411x; 1.0132x over previous
"""Trainium2 Bass kernel for nn_AttentionBlock (B=2, L=2048, D=1024, H=16).

Sharding: tensor-parallel over heads. Each of 8 cores computes 2 heads:
Wq/Wk/Wv column-sharded, Wo row-sharded; host sums the 8 partial outputs.

v3: all matmul operands bf16 (fp32 moving operands stream at 2 cycles/col on
the PE; bf16 streams at 1 — measured), software-pipelined attention loop
(scores of tile jt issue before PV of tile jt-1 so the PE never head-of-line
blocks on the exp), PE-based V transpose, batched output DMA, bf16 partials.

Per-core dataflow:
  - x^T is prepared host-side ([D, B*L] bf16, layout prep only, no math).
  - qT/kT/vT = W.T @ xT   (weight-stationary, contraction over D)
  - vT is PE-transposed to v [L, dh] blocks with a ones column appended
    (aug), so the PV matmul also produces softmax denominators for free.
  - scoresT = kT.T @ qT per (head, batch) in [Lk, Lq] layout, two heads
    row-tiled concurrently on the PE; exp on ACT (no max-subtraction:
    scores ~ N(0,1), exp is fp32-safe);
  - aT += v_aug.T @ expT accumulates attention output (+ denominator row).
  - aT is normalized via a PE-broadcast reciprocal matrix.
  - out = aT.T @ Wo (heads accumulate in PSUM), written as bf16 partials.
"""
import numpy as np
from contextlib import ExitStack

import concourse.bacc as bacc
import concourse.tile as tile
import concourse.mybir as mybir
from concourse import bass_utils
from concourse.masks import make_identity

F32 = mybir.dt.float32
BF16 = mybir.dt.bfloat16
AF = mybir.ActivationFunctionType
ALU = mybir.AluOpType

B, L, D, H, DH = 2, 2048, 1024, 16, 64
NCORES = 8
HPC = H // NCORES       # heads per core
DHC = HPC * DH          # 128 = head-dim slice per core
KT = D // 128           # 8 k-tiles over the contraction dim


def build(Lb=L, debug=False):
    """Build the per-core Bass program for per-batch seq len Lb."""
    BLb = B * Lb
    NJT = Lb // 128            # key tiles per batch
    LC = min(512, Lb)          # query-chunk width
    NLC = Lb // LC             # query chunks per batch
    PC = min(512, BLb)         # projection chunk width
    NPC = BLb // PC            # projection chunks
    VB = 130                   # v block width per (b, jt): 2 heads x (64+ones)

    nc = bacc.Bacc("TRN2", target_bir_lowering=False, debug=debug, num_devices=8)

    xT = nc.dram_tensor("xT", [D, BLb], BF16, kind="ExternalInput")
    wq = nc.dram_tensor("wq", [D, DHC], BF16, kind="ExternalInput")
    wk = nc.dram_tensor("wk", [D, DHC], BF16, kind="ExternalInput")
    wv = nc.dram_tensor("wv", [D, DHC], BF16, kind="ExternalInput")
    wo = nc.dram_tensor("wo", [DHC, D], BF16, kind="ExternalInput")
    bq = nc.dram_tensor("bq", [DHC, 1], F32, kind="ExternalInput")
    bk = nc.dram_tensor("bk", [DHC, 1], F32, kind="ExternalInput")
    bv = nc.dram_tensor("bv", [DHC, 1], F32, kind="ExternalInput")
    out = nc.dram_tensor("out", [BLb, D], BF16, kind="ExternalOutput")

    xT_v = xT.ap().rearrange("(kt p) l -> p kt l", p=128)   # [128, KT, BLb]
    wq_v = wq.ap().rearrange("(kt p) m -> p kt m", p=128)   # [128, KT, DHC]
    wk_v = wk.ap().rearrange("(kt p) m -> p kt m", p=128)
    wv_v = wv.ap().rearrange("(kt p) m -> p kt m", p=128)

    with tile.TileContext(nc) as tc, ExitStack() as ctx:
        # --- pools ---
        persist = ctx.enter_context(tc.tile_pool(name="persist", bufs=1))
        xpool = ctx.enter_context(tc.tile_pool(name="xchunk", bufs=3))
        vstage = ctx.enter_context(tc.tile_pool(name="vstage", bufs=2))
        expool = ctx.enter_context(tc.tile_pool(name="expool", bufs=3))
        denpool = ctx.enter_context(tc.tile_pool(name="denpool", bufs=2))
        outpool = ctx.enter_context(tc.tile_pool(name="outpool", bufs=2))
        # PSUM budget (8 banks): sc 2x2 + acc 1x2 + psing 2x1 -> 8
        scpool = ctx.enter_context(tc.tile_pool(name="scpool", bufs=2, space="PSUM"))
        accpool = ctx.enter_context(tc.tile_pool(name="accpool", bufs=1, space="PSUM"))
        psing = ctx.enter_context(tc.tile_pool(name="psing", bufs=2, space="PSUM"))

        # --- persistent tiles ---
        qT_sb = persist.tile([128, BLb], BF16, tag="qT")
        kT_sb = persist.tile([128, BLb], BF16, tag="kT")
        v_sb = persist.tile([128, B * NJT * VB], BF16, tag="v")
        aT_sb = [
            persist.tile([128, Lb], BF16, tag=f"aT{b}", name=f"aT{b}")
            for b in range(B)
        ]
        wq_sb = persist.tile([128, KT, DHC], BF16, tag="wq")
        wk_sb = persist.tile([128, KT, DHC], BF16, tag="wk")
        wv_sb = persist.tile([128, KT, DHC], BF16, tag="wv")
        wo_sb = persist.tile([DHC, D], BF16, tag="wo")
        bq_sb = persist.tile([DHC, 1], F32, tag="bq")
        bk_sb = persist.tile([DHC, 1], F32, tag="bk")
        bv_sb = persist.tile([DHC, 1], F32, tag="bv")
        ident = persist.tile([128, 128], BF16, tag="ident")
        ones64 = persist.tile([65, 64], BF16, tag="ones64")  # row 64 used

        # --- phase A: loads & constants ---
        nc.sync.dma_start(wq_sb[:], wq_v)
        make_identity(nc, ident[:])
        # (wk/wv/wo/biases are issued on the scalar queue inside the first
        # projection chunk so their issue overlaps the x loads)
        nc.vector.memset(ones64[:], 1.0)
        # ones-columns of v (aug denominator trick)
        v_cols = v_sb[:].rearrange("p (n c) -> p n c", c=65)
        nc.vector.memset(v_cols[:, :, 64:65], 1.0)
        # HAM warmup: the PE clock sits gated at 1.2 GHz until ~3.4us of
        # sustained activity; spin dummy matmuls on the identity tile while
        # the first x/weight DMAs are in flight so the projection phase
        # starts at the full 2.4 GHz
        warm = scpool.tile([128, 2, LC], F32, tag="sc",
                           padded_shape=[128, 2, 512], name="warm")
        for _ in range(40):
            nc.tensor.matmul(
                warm[:, 0, 0:128], ident[:], ident[:], start=True, stop=True
            )

        # --- phase emitters ---
        def emit_proj_chunk(chn):
            """Projections for rows [chn*PC, (chn+1)*PC) + v transpose."""
            cs = chn * PC
            xt = xpool.tile([128, KT, PC], BF16, tag="xt")
            if chn == 0:
                # first chunk: 2-ktile pieces on the sync queue while the
                # remaining weights/biases issue in parallel on the scalar
                # queue (idle at startup) so DMA issue time is not serial
                for kt in range(0, KT, 2):
                    nc.sync.dma_start(
                        xt[:, kt:kt + 2, :], xT_v[:, kt:kt + 2, cs:cs + PC]
                    )
                nc.scalar.dma_start(bq_sb[:], bq.ap())
                nc.scalar.dma_start(wk_sb[:], wk_v)
                nc.scalar.dma_start(bk_sb[:], bk.ap())
                nc.scalar.dma_start(wv_sb[:], wv_v)
                nc.scalar.dma_start(bv_sb[:], bv.ap())
                nc.scalar.dma_start(wo_sb[:], wo.ap())
            else:
                nc.sync.dma_start(xt[:], xT_v[:, :, cs:cs + PC])

            for w_sb, b_sb, dst in ((wq_sb, bq_sb, qT_sb), (wk_sb, bk_sb, kT_sb)):
                ps = psing.tile([128, PC], F32, tag="single")
                for kt in range(KT):
                    nc.tensor.matmul(
                        ps[:, :], w_sb[:, kt, :], xt[:, kt, :],
                        start=(kt == 0), stop=(kt == KT - 1),
                    )
                nc.vector.tensor_scalar(
                    dst[:, cs:cs + PC], ps[:, :], b_sb[:, 0:1], None, ALU.add
                )

            # v chunk -> staging (vT layout), then PE transpose into v_sb
            ps = psing.tile([128, PC], F32, tag="single")
            for kt in range(KT):
                nc.tensor.matmul(
                    ps[:, :], wv_sb[:, kt, :], xt[:, kt, :],
                    start=(kt == 0), stop=(kt == KT - 1),
                )
            vt = vstage.tile([128, PC], BF16, tag="vt")
            nc.vector.tensor_scalar(vt[:], ps[:, :], bv_sb[:, 0:1], None, ALU.add)

            for jp in range(PC // 256):
                gl = cs + jp * 256          # global row offset in [0, BLb)
                b_idx, jt = gl // Lb, (gl % Lb) // 128
                blk = (b_idx * NJT + jt) * VB
                pt = psing.tile([128, 2, 128], BF16, tag="single")
                # each [128,128] transpose covers both heads: out[j, h*64+d];
                # two consecutive key tiles share one psum tile and one copy
                for jl in range(2):
                    nc.tensor.transpose(
                        pt[:, jl, :],
                        vt[:, jp * 256 + jl * 128:jp * 256 + (jl + 1) * 128],
                        ident[:],
                    )
                vdst = v_sb[:, blk:blk + 2 * VB].rearrange(
                    "p (j h c) -> p j h c", j=2, c=65
                )
                ptv = pt[:].rearrange("p j (h c) -> p j h c", h=2)
                nc.vector.tensor_copy(vdst[:, :, :, 0:DH], ptv[:, :, :, :])

        def make_drain(b, qo, lcw, aT):
            """Deferred drain for chunk (b, qo:qo+lcw): a list of closures
            emitted one per jt-slot of the NEXT attention chunk (or flushed
            at the end). Split so no piece hogs the PE queue."""
            st = {}

            def d_copies():
                den = denpool.tile([65, 2, lcw], BF16, tag="den",
                                   padded_shape=[65, 2, 512], name="den")
                nc.vector.tensor_copy(den[64:65, :, :], aT[64:65, :, :])
                st["den"] = den
                for h in range(HPC):
                    nc.vector.tensor_copy(
                        aT_sb[b][h * DH:(h + 1) * DH, qo:qo + lcw],
                        aT[0:DH, h, :],
                    )

            def d_rep():
                rep = psing.tile([128, lcw], F32, tag="single",
                                 padded_shape=[128, 512], name="rep")
                for h in range(HPC):
                    nc.tensor.matmul(
                        rep[h * DH:(h + 1) * DH, :],
                        ones64[64:65, :],
                        st["den"][64:65, h, :],
                        start=True, stop=True,
                        tile_position=(64, h * DH),
                    )
                st["rep"] = rep
                st["rrecb"] = denpool.tile([128, lcw], BF16, tag="rrecb",
                                           padded_shape=[128, 512], name="rrecb")

            def d_recip(half):
                def f():
                    hw = lcw // 2
                    cols = slice(half * hw, (half + 1) * hw)
                    with nc.allow_low_precision(reason="softmax denom, tol 2e-2"):
                        nc.vector.reciprocal(
                            st["rrecb"][:, cols], st["rep"][:, cols]
                        )
                return f

            def d_mul(half):
                def f():
                    hw = lcw // 2
                    cols = slice(half * hw, (half + 1) * hw)
                    gcols = slice(qo + half * hw, qo + (half + 1) * hw)
                    nc.vector.tensor_mul(
                        aT_sb[b][:, gcols], aT_sb[b][:, gcols],
                        st["rrecb"][:, cols],
                    )
                    if half == 0:
                        st["ot"] = outpool.tile(
                            [128, lcw // 128, D], BF16, tag="ot",
                            padded_shape=[128, 4, D], name=f"ot{b}_{qo}"
                        )
                return f

            def d_po(ti):
                def f():
                    t = qo // 128 + ti
                    for nch in range(2):
                        po = psing.tile([128, 512], F32, tag="single")
                        nc.tensor.matmul(
                            po[:, :],
                            aT_sb[b][:, t * 128:(t + 1) * 128],
                            wo_sb[:, nch * 512:(nch + 1) * 512],
                            start=True, stop=True,
                        )
                        nc.vector.tensor_copy(
                            st["ot"][:, ti, nch * 512:(nch + 1) * 512], po[:, :]
                        )
                return f

            def d_dma():
                out_rows = out.ap()[b * Lb + qo:b * Lb + qo + lcw, :]
                nc.sync.dma_start(
                    out_rows.rearrange("(t p) d -> p t d", p=128), st["ot"][:]
                )

            skip = lambda: None
            pieces = [d_copies, d_rep, d_recip(0), d_mul(0), d_recip(1),
                      d_mul(1), skip]
            for ti in range(lcw // 128):
                pieces += [d_po(ti), skip]
            pieces.append(d_dma)
            return pieces

        def make_score_pieces(b, qo, lcw, sink):
            """Closures that pre-issue the first two score/exp tiles of the
            NEXT attention chunk inside the current chunk's tail slots, so
            ACT stays fed across the chunk boundary and through the
            interleaved projection block."""
            q0 = b * Lb + qo

            def mk(jt):
                def f():
                    k0 = b * Lb + jt * 128
                    sc = scpool.tile([128, 2, lcw], F32, tag="sc",
                                     padded_shape=[128, 2, 512], name="sc")
                    for h in range(HPC):
                        nc.tensor.matmul(
                            sc[:, h, :],
                            kT_sb[h * DH:(h + 1) * DH, k0:k0 + 128],
                            qT_sb[h * DH:(h + 1) * DH, q0:q0 + lcw],
                            start=True, stop=True,
                            tile_position=(h * DH, 0),
                        )
                    ex = expool.tile([128, 2, lcw], BF16, tag="ex",
                                     padded_shape=[128, 2, 512], name="ex")
                    nc.scalar.activation(ex[:], sc[:], AF.Exp)
                    sink.append(ex)
                return f
            return [mk(0), mk(1)]

        def emit_att_chunk(b, qo, lcw, extras, exq, tail_pieces):
            """Attention for query columns [qo, qo+lcw) of batch b, software-
            pipelined 2 deep; `exq` may arrive pre-seeded with this chunk's
            first two exp tiles (issued in the previous chunk's tail), and
            `tail_pieces` pre-issues the NEXT chunk's first scores here."""
            q0 = b * Lb + qo
            npre = len(exq)
            aT = accpool.tile([65, 2, lcw], F32, tag="acc",
                              padded_shape=[65, 2, 512], name="acc")
            for jt in range(NJT + 2):
                if npre <= jt < NJT:
                    k0 = b * Lb + jt * 128
                    sc = scpool.tile([128, 2, lcw], F32, tag="sc",
                                     padded_shape=[128, 2, 512], name="sc")
                    for h in range(HPC):
                        nc.tensor.matmul(
                            sc[:, h, :],
                            kT_sb[h * DH:(h + 1) * DH, k0:k0 + 128],
                            qT_sb[h * DH:(h + 1) * DH, q0:q0 + lcw],
                            start=True, stop=True,
                            tile_position=(h * DH, 0),
                        )
                    ex = expool.tile([128, 2, lcw], BF16, tag="ex",
                                     padded_shape=[128, 2, 512], name="ex")
                    nc.scalar.activation(ex[:], sc[:], AF.Exp)
                    exq.append(ex)
                if jt < len(extras):
                    extras[jt]()
                if jt >= NJT and jt - NJT < len(tail_pieces):
                    tail_pieces[jt - NJT]()
                if jt >= 2:
                    pj = jt - 2
                    blk = (b * NJT + pj) * VB
                    for h in range(HPC):
                        nc.tensor.matmul(
                            aT[:, h, :],
                            v_sb[:, blk + h * 65:blk + h * 65 + 65],
                            exq[pj][:, h, :],
                            start=(pj == 0), stop=(pj == NJT - 1),
                        )
            for piece in extras[NJT + 2:]:   # overflow beyond the jt slots
                piece()
            return make_drain(b, qo, lcw, aT)

        # --- main schedule: proj b=0, then attention interleaved with the
        # remaining projection chunks, drains deferred one chunk back; the
        # final chunk is split in half so its drain tail is shorter ---
        att_chunks = []
        for b in range(B):
            for lc in range(NLC):
                if b == B - 1 and lc == NLC - 1:
                    att_chunks.append((b, lc * LC, LC // 2))
                    att_chunks.append((b, lc * LC + LC // 2, LC // 2))
                else:
                    att_chunks.append((b, lc * LC, LC))
        proj_b0 = list(range(NLC))           # chunks covering batch 0 rows
        proj_rest = list(range(NLC, NPC))
        for chn in proj_b0:
            emit_proj_chunk(chn)
        deferred = []
        exq_cur = []
        for ci, (b, qo, lcw) in enumerate(att_chunks):
            if ci + 1 < len(att_chunks):
                nb, nqo, nlcw = att_chunks[ci + 1]
                exq_next = []
                tail = make_score_pieces(nb, nqo, nlcw, exq_next)
            else:
                exq_next, tail = [], []
            deferred = emit_att_chunk(b, qo, lcw, deferred, exq_cur, tail)
            if ci < len(proj_rest):
                emit_proj_chunk(proj_rest[ci])
            exq_cur = exq_next
        for piece in deferred:
            piece()

    nc.compile()
    return nc


_NC_CACHE = {}


def _get_nc(Lb=L):
    if Lb not in _NC_CACHE:
        _NC_CACHE[Lb] = build(Lb)
    return _NC_CACHE[Lb]


def make_in_maps(x, Wq, bq, Wk, bk, Wv, bv, Wo, bo, Lb=L):
    import ml_dtypes
    bf16 = ml_dtypes.bfloat16
    s = np.float32(DH ** (-0.25))
    BLb = B * Lb
    xT = np.ascontiguousarray(
        np.asarray(x, np.float32).reshape(BLb, D).T
    ).astype(bf16)
    Wq, Wk, Wv, Wo = (np.asarray(a, np.float32) for a in (Wq, Wk, Wv, Wo))
    bq, bk, bv = (np.asarray(a, np.float32) for a in (bq, bk, bv))
    in_maps = []
    for c in range(NCORES):
        hs = slice(c * DHC, (c + 1) * DHC)
        in_maps.append({
            "xT": xT,
            "wq": np.ascontiguousarray(Wq[:, hs] * s).astype(bf16),
            "wk": np.ascontiguousarray(Wk[:, hs] * s).astype(bf16),
            "wv": np.ascontiguousarray(Wv[:, hs]).astype(bf16),
            "wo": np.ascontiguousarray(Wo[hs, :]).astype(bf16),
            "bq": np.ascontiguousarray((bq[hs] * s).reshape(DHC, 1)),
            "bk": np.ascontiguousarray((bk[hs] * s).reshape(DHC, 1)),
            "bv": np.ascontiguousarray(bv[hs].reshape(DHC, 1)),
        })
    return in_maps


def kernel(x, Wq, bq, Wk, bk, Wv, bv, Wo, bo, **run_kwargs):
    x = np.asarray(x, np.float32)
    nc = _get_nc(L)
    in_maps = make_in_maps(x, Wq, bq, Wk, bk, Wv, bv, Wo, bo, L)
    res = bass_utils.run_bass_kernel_spmd(nc, in_maps, list(range(NCORES)), **run_kwargs)
    acc = np.zeros((B * L, D), np.float32)
    for r in res.results:
        acc += np.asarray(r["out"], np.float32)
    acc += np.asarray(bo, np.float32)[None, :]
    out = acc.reshape(B, L, D)
    kernel.last_results = res
    return out


# revision 24
# speedup vs baseline: 1.1474x; 1.0055x over previous
"""Trainium2 Bass kernel for nn_AttentionBlock (B=2, L=2048, D=1024, H=16).

Sharding: tensor-parallel over heads. Each of 8 cores computes 2 heads:
Wq/Wk/Wv column-sharded, Wo row-sharded; host sums the 8 partial outputs.

v3: all matmul operands bf16 (fp32 moving operands stream at 2 cycles/col on
the PE; bf16 streams at 1 — measured), software-pipelined attention loop
(scores of tile jt issue before PV of tile jt-1 so the PE never head-of-line
blocks on the exp), PE-based V transpose, batched output DMA, bf16 partials.

Per-core dataflow:
  - x^T is prepared host-side ([D, B*L] bf16, layout prep only, no math).
  - qT/kT/vT = W.T @ xT   (weight-stationary, contraction over D)
  - vT is PE-transposed to v [L, dh] blocks with a ones column appended
    (aug), so the PV matmul also produces softmax denominators for free.
  - scoresT = kT.T @ qT per (head, batch) in [Lk, Lq] layout, two heads
    row-tiled concurrently on the PE; exp on ACT (no max-subtraction:
    scores ~ N(0,1), exp is fp32-safe);
  - aT += v_aug.T @ expT accumulates attention output (+ denominator row).
  - aT is normalized via a PE-broadcast reciprocal matrix.
  - out = aT.T @ Wo (heads accumulate in PSUM), written as bf16 partials.
"""
import numpy as np
from contextlib import ExitStack

import concourse.bacc as bacc
import concourse.tile as tile
import concourse.mybir as mybir
from concourse import bass_utils
from concourse.masks import make_identity

F32 = mybir.dt.float32
BF16 = mybir.dt.bfloat16
AF = mybir.ActivationFunctionType
ALU = mybir.AluOpType

B, L, D, H, DH = 2, 2048, 1024, 16, 64
NCORES = 8
HPC = H // NCORES       # heads per core
DHC = HPC * DH          # 128 = head-dim slice per core
KT = D // 128           # 8 k-tiles over the contraction dim


def build(Lb=L, debug=False):
    """Build the per-core Bass program for per-batch seq len Lb."""
    BLb = B * Lb
    NJT = Lb // 128            # key tiles per batch
    LC = min(512, Lb)          # query-chunk width
    NLC = Lb // LC             # query chunks per batch
    PC = min(512, BLb)         # projection chunk width
    NPC = BLb // PC            # projection chunks
    VB = 130                   # v block width per (b, jt): 2 heads x (64+ones)

    nc = bacc.Bacc("TRN2", target_bir_lowering=False, debug=debug, num_devices=8)

    xT = nc.dram_tensor("xT", [D, BLb], BF16, kind="ExternalInput")
    wq = nc.dram_tensor("wq", [D, DHC], BF16, kind="ExternalInput")
    wk = nc.dram_tensor("wk", [D, DHC], BF16, kind="ExternalInput")
    wv = nc.dram_tensor("wv", [D, DHC], BF16, kind="ExternalInput")
    wo = nc.dram_tensor("wo", [DHC, D], BF16, kind="ExternalInput")
    bq = nc.dram_tensor("bq", [DHC, 1], F32, kind="ExternalInput")
    bk = nc.dram_tensor("bk", [DHC, 1], F32, kind="ExternalInput")
    bv = nc.dram_tensor("bv", [DHC, 1], F32, kind="ExternalInput")
    out = nc.dram_tensor("out", [BLb, D], BF16, kind="ExternalOutput")

    xT_v = xT.ap().rearrange("(kt p) l -> p kt l", p=128)   # [128, KT, BLb]
    wq_v = wq.ap().rearrange("(kt p) m -> p kt m", p=128)   # [128, KT, DHC]
    wk_v = wk.ap().rearrange("(kt p) m -> p kt m", p=128)
    wv_v = wv.ap().rearrange("(kt p) m -> p kt m", p=128)

    with tile.TileContext(nc) as tc, ExitStack() as ctx:
        # --- pools ---
        persist = ctx.enter_context(tc.tile_pool(name="persist", bufs=1))
        xpool = ctx.enter_context(tc.tile_pool(name="xchunk", bufs=3))
        vstage = ctx.enter_context(tc.tile_pool(name="vstage", bufs=2))
        expool = ctx.enter_context(tc.tile_pool(name="expool", bufs=4))
        denpool = ctx.enter_context(tc.tile_pool(name="denpool", bufs=2))
        outpool = ctx.enter_context(tc.tile_pool(name="outpool", bufs=2))
        # PSUM budget (8 banks): sc 2x2 + acc 1x2 + psing 2x1 -> 8
        scpool = ctx.enter_context(tc.tile_pool(name="scpool", bufs=2, space="PSUM"))
        accpool = ctx.enter_context(tc.tile_pool(name="accpool", bufs=1, space="PSUM"))
        psing = ctx.enter_context(tc.tile_pool(name="psing", bufs=2, space="PSUM"))

        # --- persistent tiles ---
        qT_sb = persist.tile([128, BLb], BF16, tag="qT")
        kT_sb = persist.tile([128, BLb], BF16, tag="kT")
        v_sb = persist.tile([128, B * NJT * VB], BF16, tag="v")
        aT_sb = [
            persist.tile([128, Lb], BF16, tag=f"aT{b}", name=f"aT{b}")
            for b in range(B)
        ]
        wq_sb = persist.tile([128, KT, DHC], BF16, tag="wq")
        wk_sb = persist.tile([128, KT, DHC], BF16, tag="wk")
        wv_sb = persist.tile([128, KT, DHC], BF16, tag="wv")
        wo_sb = persist.tile([DHC, D], BF16, tag="wo")
        bq_sb = persist.tile([DHC, 1], F32, tag="bq")
        bk_sb = persist.tile([DHC, 1], F32, tag="bk")
        bv_sb = persist.tile([DHC, 1], F32, tag="bv")
        ident = persist.tile([128, 128], BF16, tag="ident")
        ones64 = persist.tile([65, 64], BF16, tag="ones64")  # row 64 used

        # --- phase A: loads & constants ---
        nc.sync.dma_start(wq_sb[:], wq_v)
        make_identity(nc, ident[:])
        # (wk/wv/wo/biases are issued on the scalar queue inside the first
        # projection chunk so their issue overlaps the x loads)
        nc.vector.memset(ones64[:], 1.0)
        # ones-columns of v (aug denominator trick)
        v_cols = v_sb[:].rearrange("p (n c) -> p n c", c=65)
        nc.vector.memset(v_cols[:, :, 64:65], 1.0)
        # HAM warmup: the PE clock sits gated at 1.2 GHz until ~3.4us of
        # sustained activity; spin dummy matmuls on the identity tile while
        # the first x/weight DMAs are in flight so the projection phase
        # starts at the full 2.4 GHz
        warm = scpool.tile([128, 2, LC], F32, tag="sc",
                           padded_shape=[128, 2, 512], name="warm")
        for _ in range(40):
            nc.tensor.matmul(
                warm[:, 0, 0:128], ident[:], ident[:], start=True, stop=True
            )

        # --- phase emitters ---
        def emit_proj_chunk(chn):
            """Projections for rows [chn*PC, (chn+1)*PC) + v transpose."""
            cs = chn * PC
            xt = xpool.tile([128, KT, PC], BF16, tag="xt")
            if chn == 0:
                # first chunk: 2-ktile pieces on the sync queue while the
                # remaining weights/biases issue in parallel on the scalar
                # queue (idle at startup) so DMA issue time is not serial
                for kt in range(0, KT, 2):
                    nc.sync.dma_start(
                        xt[:, kt:kt + 2, :], xT_v[:, kt:kt + 2, cs:cs + PC]
                    )
                nc.scalar.dma_start(bq_sb[:], bq.ap())
                nc.scalar.dma_start(wk_sb[:], wk_v)
                nc.scalar.dma_start(bk_sb[:], bk.ap())
                nc.scalar.dma_start(wv_sb[:], wv_v)
                nc.scalar.dma_start(bv_sb[:], bv.ap())
                nc.scalar.dma_start(wo_sb[:], wo.ap())
            else:
                nc.sync.dma_start(xt[:], xT_v[:, :, cs:cs + PC])

            for w_sb, b_sb, dst in ((wq_sb, bq_sb, qT_sb), (wk_sb, bk_sb, kT_sb)):
                ps = psing.tile([128, PC], F32, tag="single")
                for kt in range(KT):
                    nc.tensor.matmul(
                        ps[:, :], w_sb[:, kt, :], xt[:, kt, :],
                        start=(kt == 0), stop=(kt == KT - 1),
                    )
                nc.vector.tensor_scalar(
                    dst[:, cs:cs + PC], ps[:, :], b_sb[:, 0:1], None, ALU.add
                )

            # v chunk -> staging (vT layout), then PE transpose into v_sb
            ps = psing.tile([128, PC], F32, tag="single")
            for kt in range(KT):
                nc.tensor.matmul(
                    ps[:, :], wv_sb[:, kt, :], xt[:, kt, :],
                    start=(kt == 0), stop=(kt == KT - 1),
                )
            vt = vstage.tile([128, PC], BF16, tag="vt")
            nc.vector.tensor_scalar(vt[:], ps[:, :], bv_sb[:, 0:1], None, ALU.add)

            for jp in range(PC // 256):
                gl = cs + jp * 256          # global row offset in [0, BLb)
                b_idx, jt = gl // Lb, (gl % Lb) // 128
                blk = (b_idx * NJT + jt) * VB
                pt = psing.tile([128, 2, 128], BF16, tag="single")
                # each [128,128] transpose covers both heads: out[j, h*64+d];
                # two consecutive key tiles share one psum tile and one copy
                for jl in range(2):
                    nc.tensor.transpose(
                        pt[:, jl, :],
                        vt[:, jp * 256 + jl * 128:jp * 256 + (jl + 1) * 128],
                        ident[:],
                    )
                vdst = v_sb[:, blk:blk + 2 * VB].rearrange(
                    "p (j h c) -> p j h c", j=2, c=65
                )
                ptv = pt[:].rearrange("p j (h c) -> p j h c", h=2)
                nc.vector.tensor_copy(vdst[:, :, :, 0:DH], ptv[:, :, :, :])

        def make_drain(b, qo, lcw, aT):
            """Deferred drain for chunk (b, qo:qo+lcw): a list of closures
            emitted one per jt-slot of the NEXT attention chunk (or flushed
            at the end). Split so no piece hogs the PE queue."""
            st = {}

            def d_copies():
                den = denpool.tile([65, 2, lcw], BF16, tag="den",
                                   padded_shape=[65, 2, 512], name="den")
                nc.vector.tensor_copy(den[64:65, :, :], aT[64:65, :, :])
                st["den"] = den
                for h in range(HPC):
                    nc.vector.tensor_copy(
                        aT_sb[b][h * DH:(h + 1) * DH, qo:qo + lcw],
                        aT[0:DH, h, :],
                    )

            def d_rep():
                rep = psing.tile([128, lcw], F32, tag="single",
                                 padded_shape=[128, 512], name="rep")
                for h in range(HPC):
                    nc.tensor.matmul(
                        rep[h * DH:(h + 1) * DH, :],
                        ones64[64:65, :],
                        st["den"][64:65, h, :],
                        start=True, stop=True,
                        tile_position=(64, h * DH),
                    )
                st["rep"] = rep
                st["rrecb"] = denpool.tile([128, lcw], BF16, tag="rrecb",
                                           padded_shape=[128, 512], name="rrecb")

            def d_recip(half):
                def f():
                    hw = lcw // 2
                    cols = slice(half * hw, (half + 1) * hw)
                    with nc.allow_low_precision(reason="softmax denom, tol 2e-2"):
                        nc.vector.reciprocal(
                            st["rrecb"][:, cols], st["rep"][:, cols]
                        )
                return f

            def d_mul(half):
                def f():
                    hw = lcw // 2
                    cols = slice(half * hw, (half + 1) * hw)
                    gcols = slice(qo + half * hw, qo + (half + 1) * hw)
                    nc.vector.tensor_mul(
                        aT_sb[b][:, gcols], aT_sb[b][:, gcols],
                        st["rrecb"][:, cols],
                    )
                    if half == 0:
                        st["ot"] = outpool.tile(
                            [128, lcw // 128, D], BF16, tag="ot",
                            padded_shape=[128, 4, D], name=f"ot{b}_{qo}"
                        )
                return f

            def d_po(ti):
                def f():
                    t = qo // 128 + ti
                    for nch in range(2):
                        po = psing.tile([128, 512], F32, tag="single")
                        nc.tensor.matmul(
                            po[:, :],
                            aT_sb[b][:, t * 128:(t + 1) * 128],
                            wo_sb[:, nch * 512:(nch + 1) * 512],
                            start=True, stop=True,
                        )
                        nc.vector.tensor_copy(
                            st["ot"][:, ti, nch * 512:(nch + 1) * 512], po[:, :]
                        )
                return f

            def d_dma():
                out_rows = out.ap()[b * Lb + qo:b * Lb + qo + lcw, :]
                nc.sync.dma_start(
                    out_rows.rearrange("(t p) d -> p t d", p=128), st["ot"][:]
                )

            skip = lambda: None
            pieces = [d_copies, d_rep, d_recip(0), d_mul(0), d_recip(1),
                      d_mul(1), skip]
            for ti in range(lcw // 128):
                pieces += [d_po(ti), skip]
            pieces.append(d_dma)
            return pieces

        def make_score_pieces(b, qo, lcw, sink):
            """Closures that pre-issue the first two score/exp tiles of the
            NEXT attention chunk inside the current chunk's tail slots, so
            ACT stays fed across the chunk boundary and through the
            interleaved projection block."""
            q0 = b * Lb + qo

            def mk(jt):
                def f():
                    k0 = b * Lb + jt * 128
                    sc = scpool.tile([128, 2, lcw], F32, tag="sc",
                                     padded_shape=[128, 2, 512], name="sc")
                    for h in range(HPC):
                        nc.tensor.matmul(
                            sc[:, h, :],
                            kT_sb[h * DH:(h + 1) * DH, k0:k0 + 128],
                            qT_sb[h * DH:(h + 1) * DH, q0:q0 + lcw],
                            start=True, stop=True,
                            tile_position=(h * DH, 0),
                        )
                    ex = expool.tile([128, 2, lcw], BF16, tag="ex",
                                     padded_shape=[128, 2, 512], name="ex")
                    nc.scalar.activation(ex[:], sc[:], AF.Exp)
                    sink.append(ex)
                return f
            return [mk(0), mk(1)]

        def emit_att_chunk(b, qo, lcw, extras, exq, tail_pieces):
            """Attention for query columns [qo, qo+lcw) of batch b, software-
            pipelined 2 deep; `exq` may arrive pre-seeded with this chunk's
            first two exp tiles (issued in the previous chunk's tail), and
            `tail_pieces` pre-issues the NEXT chunk's first scores here."""
            q0 = b * Lb + qo
            npre = len(exq)
            aT = accpool.tile([65, 2, lcw], F32, tag="acc",
                              padded_shape=[65, 2, 512], name="acc")
            for jt in range(NJT + 2):
                if npre <= jt < NJT:
                    k0 = b * Lb + jt * 128
                    sc = scpool.tile([128, 2, lcw], F32, tag="sc",
                                     padded_shape=[128, 2, 512], name="sc")
                    for h in range(HPC):
                        nc.tensor.matmul(
                            sc[:, h, :],
                            kT_sb[h * DH:(h + 1) * DH, k0:k0 + 128],
                            qT_sb[h * DH:(h + 1) * DH, q0:q0 + lcw],
                            start=True, stop=True,
                            tile_position=(h * DH, 0),
                        )
                    ex = expool.tile([128, 2, lcw], BF16, tag="ex",
                                     padded_shape=[128, 2, 512], name="ex")
                    nc.scalar.activation(ex[:], sc[:], AF.Exp)
                    exq.append(ex)
                if jt < len(extras):
                    extras[jt]()
                if jt >= NJT and jt - NJT < len(tail_pieces):
                    tail_pieces[jt - NJT]()
                if jt >= 2:
                    pj = jt - 2
                    blk = (b * NJT + pj) * VB
                    for h in range(HPC):
                        nc.tensor.matmul(
                            aT[:, h, :],
                            v_sb[:, blk + h * 65:blk + h * 65 + 65],
                            exq[pj][:, h, :],
                            start=(pj == 0), stop=(pj == NJT - 1),
                        )
            for piece in extras[NJT + 2:]:   # overflow beyond the jt slots
                piece()
            return make_drain(b, qo, lcw, aT)

        # --- main schedule: proj b=0, then attention interleaved with the
        # remaining projection chunks, drains deferred one chunk back; the
        # final chunk is split in half so its drain tail is shorter ---
        att_chunks = []
        for b in range(B):
            for lc in range(NLC):
                if b == B - 1 and lc == NLC - 1:
                    att_chunks.append((b, lc * LC, LC // 2))
                    att_chunks.append((b, lc * LC + LC // 2, LC // 2))
                else:
                    att_chunks.append((b, lc * LC, LC))
        proj_b0 = list(range(NLC))           # chunks covering batch 0 rows
        proj_rest = list(range(NLC, NPC))
        for chn in proj_b0:
            emit_proj_chunk(chn)
        deferred = []
        exq_cur = []
        for ci, (b, qo, lcw) in enumerate(att_chunks):
            if ci + 1 < len(att_chunks):
                nb, nqo, nlcw = att_chunks[ci + 1]
                exq_next = []
                tail = make_score_pieces(nb, nqo, nlcw, exq_next)
            else:
                exq_next, tail = [], []
            deferred = emit_att_chunk(b, qo, lcw, deferred, exq_cur, tail)
            if ci < len(proj_rest):
                emit_proj_chunk(proj_rest[ci])
            exq_cur = exq_next
        for piece in deferred:
            piece()

    nc.compile()
    return nc


_NC_CACHE = {}


def _get_nc(Lb=L):
    if Lb not in _NC_CACHE:
        _NC_CACHE[Lb] = build(Lb)
    return _NC_CACHE[Lb]


def make_in_maps(x, Wq, bq, Wk, bk, Wv, bv, Wo, bo, Lb=L):
    import ml_dtypes
    bf16 = ml_dtypes.bfloat16
    s = np.float32(DH ** (-0.25))
    BLb = B * Lb
    xT = np.ascontiguousarray(
        np.asarray(x, np.float32).reshape(BLb, D).T
    ).astype(bf16)
    Wq, Wk, Wv, Wo = (np.asarray(a, np.float32) for a in (Wq, Wk, Wv, Wo))
    bq, bk, bv = (np.asarray(a, np.float32) for a in (bq, bk, bv))
    in_maps = []
    for c in range(NCORES):
        hs = slice(c * DHC, (c + 1) * DHC)
        in_maps.append({
            "xT": xT,
            "wq": np.ascontiguousarray(Wq[:, hs] * s).astype(bf16),
            "wk": np.ascontiguousarray(Wk[:, hs] * s).astype(bf16),
            "wv": np.ascontiguousarray(Wv[:, hs]).astype(bf16),
            "wo": np.ascontiguousarray(Wo[hs, :]).astype(bf16),
            "bq": np.ascontiguousarray((bq[hs] * s).reshape(DHC, 1)),
            "bk": np.ascontiguousarray((bk[hs] * s).reshape(DHC, 1)),
            "bv": np.ascontiguousarray(bv[hs].reshape(DHC, 1)),
        })
    return in_maps


def kernel(x, Wq, bq, Wk, bk, Wv, bv, Wo, bo, **run_kwargs):
    x = np.asarray(x, np.float32)
    nc = _get_nc(L)
    in_maps = make_in_maps(x, Wq, bq, Wk, bk, Wv, bv, Wo, bo, L)
    res = bass_utils.run_bass_kernel_spmd(nc, in_maps, list(range(NCORES)), **run_kwargs)
    acc = np.zeros((B * L, D), np.float32)
    for r in res.results:
        acc += np.asarray(r["out"], np.float32)
    acc += np.asarray(bo, np.float32)[None, :]
    out = acc.reshape(B, L, D)
    kernel.last_results = res
    return out
